# revision 25
# baseline (speedup 1.0000x reference)
"""TRN2 Bass kernel for nn_CoreAttention_34875134444341.

Strategy (8 NeuronCores):
  - Data-parallel over batch (4) x causal-balanced query-row split (2).
  - TILE_R=128 zig-zag row tiles (8 slots/core) balance causal work to 72
    key-block units/core.
  - K/V projections are token-split across each core pair: every core
    computes K/V for half the sequence, pairs exchange halves with a DRAM
    AllGather that overlaps the Q projection.
  - All matmuls bf16. Head-QUAD fusion: the 4 query heads sharing one KV
    head are processed by single [128,512] matmuls (scores, exp, PV, sum).
  - Attention runs "transposed" (keys/dk on the partition axis), q/k/v/attn
    all SBUF-resident (no DRAM scratch).
  - Softmax denominators via a ones[128,128] stationary matmul that yields
    partition-broadcast sums; normalization = reciprocal_approx_fast + one
    tensor_tensor multiply on DVE.
  - Software pipeline with lookahead-2 on the score matmuls keeps the PE
    busy while the Activation engine computes exp.
  - Final Wo matmul row-parallel (no cross-core reduction).
"""

import sys

sys.path.insert(0, "/opt/trn_rl_repo")

import numpy as np
import ml_dtypes

B, S, D = 4, 2048, 2048
H, HKV, DK = 16, 4, 128
RQ = RKV = 512
GROUP = H // HKV
P = 128
SH = S // 2  # tokens whose K/V are computed locally

TILE_R = 128
KB = 128
NSLOT = 8
NB_SCHED = [16, 14, 12, 10, 8, 6, 4, 2]
TILE_ASSIGN = {
    0: [15, 12, 11, 8, 7, 4, 3, 0],
    1: [14, 13, 10, 9, 6, 5, 2, 1],
}
ROWS_PER_CORE = NSLOT * TILE_R  # 1024

_CACHE = {}
TRACE = False
LAST_RESULT = None


def _rows_sched(parity):
    return np.concatenate(
        [np.arange(t * TILE_R, (t + 1) * TILE_R) for t in TILE_ASSIGN[parity]]
    )


def _make_mask(parity):
    """[128 keys, 8 slots, 2 j, 512 (4 heads x 128 rows)].

    j=0 -> block nb-2, j=1 -> block nb-1 of each slot's schedule.
    Pattern per (slot, j): zeros (fully visible), triangle (diagonal
    block), or full -1e30 (pad block past this parity's causal need).
    """
    k = np.arange(KB)
    r = np.arange(TILE_R)
    tri = np.where(k[:, None] > r[None, :], np.float32(-1e30), np.float32(0.0))
    tri4 = np.tile(tri, (1, 4))  # same per head in the quad
    full = np.full((KB, 4 * TILE_R), -1e30, np.float32)
    zeros = np.zeros((KB, 4 * TILE_R), np.float32)

    m = np.zeros((KB, NSLOT, 2, 4 * TILE_R), np.float32)
    for s in range(NSLOT):
        t = TILE_ASSIGN[parity][s]
        nb = NB_SCHED[s]
        need = t + 1
        for j, b in enumerate((nb - 2, nb - 1)):
            if b == t:
                m[:, s, j] = tri4
            elif b >= need:
                m[:, s, j] = full
            else:
                m[:, s, j] = zeros
    return m


def _build_nc():
    import concourse.tile as tile
    from concourse import bacc, mybir

    f32 = mybir.dt.float32
    bf16 = mybir.dt.bfloat16
    Exp = mybir.ActivationFunctionType.Exp
    Mult = mybir.AluOpType.mult
    Add = mybir.AluOpType.add

    nc = bacc.Bacc("TRN2", target_bir_lowering=False, debug=False, num_devices=8)

    xq = nc.dram_tensor("xq", [D, ROWS_PER_CORE], bf16, kind="ExternalInput")
    xkv = nc.dram_tensor("xkv", [D, SH], bf16, kind="ExternalInput")
    wq1 = nc.dram_tensor("wq1", [D, RQ], bf16, kind="ExternalInput")
    wq2 = nc.dram_tensor("wq2", [RQ, H * DK], bf16, kind="ExternalInput")
    wk1 = nc.dram_tensor("wk1", [D, RKV], bf16, kind="ExternalInput")
    wk2 = nc.dram_tensor("wk2", [RKV, HKV * DK], bf16, kind="ExternalInput")
    wv1 = nc.dram_tensor("wv1", [D, RKV], bf16, kind="ExternalInput")
    wv2 = nc.dram_tensor("wv2", [RKV, HKV * DK], bf16, kind="ExternalInput")
    wo = nc.dram_tensor("wo", [D, D], bf16, kind="ExternalInput")
    maskin = nc.dram_tensor(
        "maskin", [KB, NSLOT, 2, 4 * TILE_R], bf16, kind="ExternalInput"
    )
    ones_in = nc.dram_tensor("ones_in", [P, P], bf16, kind="ExternalInput")
    out = nc.dram_tensor("out", [ROWS_PER_CORE, D], f32, kind="ExternalOutput")

    kT_loc = nc.dram_tensor("kT_loc", [P, HKV, SH], bf16)
    v_loc = nc.dram_tensor("v_loc", [P, SH // P, HKV * DK], bf16)
    kT_all = nc.dram_tensor("kT_all", [2, P, HKV, SH], bf16)
    v_all = nc.dram_tensor("v_all", [2, P, SH // P, HKV * DK], bf16)

    xkv_t = xkv.rearrange("(dc p) s -> p dc s", p=P)  # [128, 16, 1024]
    xq_t = xq.rearrange("(dc p) r -> p dc r", p=P)  # [128, 16, 1024]
    wq1_t = wq1.rearrange("(dc p) r -> p dc r", p=P)  # [128, 16, 512]
    wk1_t = wk1.rearrange("(dc p) r -> p dc r", p=P)
    wv1_t = wv1.rearrange("(dc p) r -> p dc r", p=P)
    wq2_t = wq2.rearrange("(rc p) h -> p rc h", p=P)  # [128, 4, 2048]
    wk2_t = wk2.rearrange("(rc p) h -> p rc h", p=P)  # [128, 4, 512]
    wv2_t = wv2.rearrange("(rc p) h -> p rc h", p=P)
    wo_t = wo.rearrange("(hc p) o -> p hc o", p=P)  # [128, 16, 2048]
    kT_all_r = kT_all.rearrange("h p c t -> p h c t")  # [128, 2, 4, 1024]
    v_all_r = v_all.rearrange("h p b d -> p h b d")  # [128, 2, 8, 512]

    groups = [[0, 1], [2, 3], [4, 5], [6, 7]]

    def make_copier():
        state = {"i": 0}

        def cp(dst, src):
            if state["i"] % 2 == 0:
                nc.vector.tensor_copy(dst, src)
            else:
                nc.scalar.copy(dst, src)
            state["i"] += 1

        return cp

    with tile.TileContext(nc) as tc:
        with tc.tile_pool(name="persist", bufs=1) as persist:
            qT_sb = persist.tile([P, NSLOT, H * TILE_R], bf16)
            kT_sb = persist.tile([P, HKV, S], bf16)
            v_sb = persist.tile([P, S // P, HKV * DK], bf16)

            # ---- long-lived projection pools (LIFO stack) ---------------
            kv_w_cm = tc.tile_pool(name="kv_w", bufs=1)
            kv_w = kv_w_cm.__enter__()
            kv_x_cm = tc.tile_pool(name="kv_x", bufs=1)
            kv_x = kv_x_cm.__enter__()
            p1_a_cm = tc.tile_pool(name="p1_a", bufs=1)
            p1_a = p1_a_cm.__enter__()
            p1_b_cm = tc.tile_pool(name="p1_b", bufs=1)
            p1_b = p1_b_cm.__enter__()

            wk1_sb = kv_w.tile([P, 16, RKV], bf16)
            nc.sync.dma_start(wk1_sb[:], wk1_t)
            wv1_sb = kv_w.tile([P, 16, RKV], bf16)
            nc.sync.dma_start(wv1_sb[:], wv1_t)
            wk2_sb = kv_w.tile([P, 4, HKV * DK], bf16)
            nc.sync.dma_start(wk2_sb[:], wk2_t)
            wv2_sb = kv_w.tile([P, 4, HKV * DK], bf16)
            nc.sync.dma_start(wv2_sb[:], wv2_t)
            xt0 = kv_x.tile([P, 16, 512], bf16, tag="xt")
            nc.sync.dma_start(xt0[:], xkv_t[:, :, 0:512])

            wq1_sb = p1_b.tile([P, 16, RQ], bf16)
            nc.sync.dma_start(wq1_sb[:], wq1_t)
            xqs = []
            for tcn in range(2):
                xqt = p1_b.tile([P, 16, 512], bf16, tag=f"xq{tcn}")
                nc.sync.dma_start(xqt[:], xq_t[:, :, tcn * 512 : (tcn + 1) * 512])
                xqs.append(xqt)
            wq2_sb = p1_a.tile([P, 4, H * DK], bf16)
            q1t = p1_a.tile([P, 4, ROWS_PER_CORE], bf16)

            # ---- Phase 2: K/V projections for the local token half ------
            cp2 = make_copier()
            with (
                tc.tile_pool(name="kv_mid", bufs=1) as kv_mid,
                tc.tile_pool(name="kv_ps1", bufs=3, space="PSUM") as kv_ps1,
                tc.tile_pool(name="kv_ps2", bufs=3, space="PSUM") as kv_ps2,
            ):
                for tcn in range(2):
                    if tcn == 0:
                        xt = xt0
                    else:
                        xt = kv_x.tile([P, 16, 512], bf16, tag="xt")
                        nc.sync.dma_start(
                            xt[:], xkv_t[:, :, tcn * 512 : (tcn + 1) * 512]
                        )
                    midk = kv_mid.tile([P, 4, 512], bf16, tag="midk")
                    midv = kv_mid.tile([P, 4, 512], bf16, tag="midv")
                    for w1_sb, mid in ((wk1_sb, midk), (wv1_sb, midv)):
                        for rc in range(4):
                            ps_1 = kv_ps1.tile([P, 512], f32, tag="ps1")
                            for dc in range(16):
                                nc.tensor.matmul(
                                    ps_1[:],
                                    w1_sb[:, dc, rc * P : (rc + 1) * P],
                                    xt[:, dc],
                                    start=(dc == 0),
                                    stop=(dc == 15),
                                )
                            cp2(mid[:, rc], ps_1[:])
                    for hc in range(HKV):
                        ps_2 = kv_ps2.tile([P, 512], f32, tag="ps2")
                        for rc in range(4):
                            nc.tensor.matmul(
                                ps_2[:],
                                wk2_sb[:, rc, hc * P : (hc + 1) * P],
                                midk[:, rc],
                                start=(rc == 0),
                                stop=(rc == 3),
                            )
                        cp2(kT_sb[:, hc, tcn * 512 : (tcn + 1) * 512], ps_2[:])
                    for i in range(4):
                        ps_2 = kv_ps2.tile([P, 512], f32, tag="ps2")
                        for rc in range(4):
                            nc.tensor.matmul(
                                ps_2[:],
                                midv[:, rc, i * P : (i + 1) * P],
                                wv2_sb[:, rc],
                                start=(rc == 0),
                                stop=(rc == 3),
                            )
                        cp2(v_sb[:, tcn * 4 + i], ps_2[:])

            # bounce local halves to DRAM and exchange within the pair
            nc.sync.dma_start(kT_loc[:], kT_sb[:, :, 0:SH])
            nc.sync.dma_start(v_loc[:], v_sb[:, 0 : SH // P, :])
            nc.gpsimd.collective_compute(
                "AllGather",
                mybir.AluOpType.bypass,
                replica_groups=groups,
                ins=[kT_loc[:].opt()],
                outs=[kT_all[:].opt()],
            )
            nc.gpsimd.collective_compute(
                "AllGather",
                mybir.AluOpType.bypass,
                replica_groups=groups,
                ins=[v_loc[:].opt()],
                outs=[v_all[:].opt()],
            )

            # ---- Phase 1: Q projection (overlaps the AllGather) ---------
            cp1 = make_copier()
            nc.sync.dma_start(wq2_sb[:], wq2_t)
            with tc.tile_pool(name="q_ps1", bufs=4, space="PSUM") as q_ps:
                for tcn in range(2):
                    for rc in range(4):
                        ps_q = q_ps.tile([P, 512], f32, tag="psq1")
                        for dc in range(16):
                            nc.tensor.matmul(
                                ps_q[:],
                                wq1_sb[:, dc, rc * P : (rc + 1) * P],
                                xqs[tcn][:, dc],
                                start=(dc == 0),
                                stop=(dc == 15),
                            )
                        cp1(q1t[:, rc, tcn * 512 : (tcn + 1) * 512], ps_q[:])
            with tc.tile_pool(name="q_ps2", bufs=4, space="PSUM") as q_ps2:
                for h in range(H):
                    for tcn in range(2):
                        ps_qT = q_ps2.tile([P, 512], f32, tag="psq2")
                        for rc in range(4):
                            nc.tensor.matmul(
                                ps_qT[:],
                                wq2_sb[:, rc, h * P : (h + 1) * P],
                                q1t[:, rc, tcn * 512 : (tcn + 1) * 512],
                                start=(rc == 0),
                                stop=(rc == 3),
                            )
                        off = (h // GROUP) * GROUP * TILE_R + (h % GROUP) * TILE_R
                        cp1(
                            qT_sb[:, 4 * tcn : 4 * tcn + 4, off : off + TILE_R],
                            ps_qT[:],
                        )
            p1_b_cm.__exit__(None, None, None)
            p1_a_cm.__exit__(None, None, None)
            kv_x_cm.__exit__(None, None, None)
            kv_w_cm.__exit__(None, None, None)

            # gather results back into the resident K/V tiles
            for half in range(2):
                nc.sync.dma_start(
                    kT_sb[:, :, half * SH : (half + 1) * SH], kT_all_r[:, half]
                )
                nc.sync.dma_start(
                    v_sb[:, half * (SH // P) : (half + 1) * (SH // P), :],
                    v_all_r[:, half],
                )

            # ---- Phase 3 + 4 pools --------------------------------------
            with tc.tile_pool(name="p3_keep", bufs=1) as p3_keep:
                attn_sb = p3_keep.tile([P, NSLOT, H * TILE_R], bf16)
                mask_sb = p3_keep.tile([P, NSLOT, 2, 4 * TILE_R], bf16)
                nc.sync.dma_start(mask_sb[:], maskin[:])
                ones_sb = p3_keep.tile([P, P], bf16)
                nc.sync.dma_start(ones_sb[:], ones_in[:])

                wo_pool_cm = tc.tile_pool(name="wo_w", bufs=2)
                wo_w = wo_pool_cm.__enter__()
                wo_tiles = {}
                for oc in range(2):
                    wt = wo_w.tile([P, 16, 512], bf16, tag="woc")
                    nc.sync.dma_start(wt[:], wo_t[:, :, oc * 512 : (oc + 1) * 512])
                    wo_tiles[oc] = wt

                # ---- Phase 3: attention (software-pipelined) ------------
                items = []
                for s in range(NSLOT):
                    nb = NB_SCHED[s]
                    for g in range(HKV):
                        for b in range(nb):
                            j = b - (nb - 2)
                            items.append(
                                (s, g, b, b == 0, b == nb - 1, j if j >= 0 else None)
                            )
                n_items = len(items)

                with (
                    tc.tile_pool(name="at_e", bufs=4) as at_e,
                    tc.tile_pool(name="at_r", bufs=2) as at_r,
                    tc.tile_pool(name="at_ps_sc", bufs=3, space="PSUM") as at_ps_sc,
                    tc.tile_pool(name="at_ps_at", bufs=2, space="PSUM") as at_ps_at,
                    tc.tile_pool(name="at_ps_sum", bufs=2, space="PSUM") as at_ps_sum,
                ):
                    sc_tiles = {}
                    acc = {}

                    def emit_sc(i):
                        s, g, b, first, last, mj = items[i]
                        ps_sc = at_ps_sc.tile([P, 4 * TILE_R], f32, tag="sc")
                        nc.tensor.matmul(
                            ps_sc[:],
                            kT_sb[:, g, b * KB : (b + 1) * KB],
                            qT_sb[
                                :, s, g * GROUP * TILE_R : (g + 1) * GROUP * TILE_R
                            ],
                            start=True,
                            stop=True,
                        )
                        if mj is not None:
                            nc.vector.tensor_tensor(
                                ps_sc[:], ps_sc[:], mask_sb[:, s, mj], Add
                            )
                        sc_tiles[i] = ps_sc

                    def emit_exp(i):
                        e_sb = at_e.tile([P, 4 * TILE_R], bf16, tag="e")
                        nc.scalar.activation(e_sb[:], sc_tiles.pop(i)[:], Exp)
                        return e_sb

                    def emit_pv(i, e_sb):
                        s, g, b, first, last, mj = items[i]
                        if first:
                            acc[(s, g)] = (
                                at_ps_at.tile(
                                    [P, 4 * TILE_R], f32, tag="at", name="ps_at"
                                ),
                                at_ps_sum.tile(
                                    [P, 4 * TILE_R], f32, tag="sum", name="ps_sum"
                                ),
                            )
                        ps_at, ps_sum = acc[(s, g)]
                        nc.tensor.matmul(
                            ps_at[:],
                            v_sb[:, b, g * DK : (g + 1) * DK],
                            e_sb[:],
                            start=first,
                            stop=last,
                        )
                        nc.tensor.matmul(
                            ps_sum[:],
                            ones_sb[:],
                            e_sb[:],
                            start=first,
                            stop=last,
                        )
                        if last:
                            ps_at, ps_sum = acc.pop((s, g))
                            rec = at_r.tile([P, 4 * TILE_R], f32, tag="rec")
                            nc.vector.reciprocal_approx_fast(rec[:], ps_sum[:])
                            nc.vector.tensor_tensor(
                                attn_sb[
                                    :,
                                    s,
                                    g * GROUP * TILE_R : (g + 1) * GROUP * TILE_R,
                                ],
                                ps_at[:],
                                rec[:],
                                Mult,
                            )

                    emit_sc(0)
                    if n_items > 1:
                        emit_sc(1)
                    for i in range(n_items):
                        e_sb = emit_exp(i)
                        if i + 2 < n_items:
                            emit_sc(i + 2)
                        emit_pv(i, e_sb)

                # ---- Phase 4: Wo ----------------------------------------
                with (
                    tc.tile_pool(name="wo_out", bufs=3) as wo_out,
                    tc.tile_pool(name="wo_ps", bufs=3, space="PSUM") as wo_ps,
                ):
                    for oc in range(4):
                        if oc in wo_tiles:
                            wo_sb = wo_tiles[oc]
                        else:
                            wo_sb = wo_w.tile([P, 16, 512], bf16, tag="woc")
                            nc.sync.dma_start(
                                wo_sb[:], wo_t[:, :, oc * 512 : (oc + 1) * 512]
                            )
                        for s in range(NSLOT):
                            ps_o = wo_ps.tile([P, 512], f32, tag="o")
                            for hc in range(16):
                                nc.tensor.matmul(
                                    ps_o[:],
                                    attn_sb[:, s, hc * TILE_R : (hc + 1) * TILE_R],
                                    wo_sb[:, hc],
                                    start=(hc == 0),
                                    stop=(hc == 15),
                                )
                            o_sb = wo_out.tile([P, 512], f32, tag="osb")
                            nc.vector.tensor_copy(o_sb[:], ps_o[:])
                            nc.sync.dma_start(
                                out[s * P : (s + 1) * P, oc * 512 : (oc + 1) * 512],
                                o_sb[:],
                            )
                wo_pool_cm.__exit__(None, None, None)

    nc.finalize()
    return nc


def kernel(x, Wq1, Wq2, Wk1, Wk2, Wv1, Wv2, Wo):
    global LAST_RESULT
    from concourse.bass_utils import run_bass_kernel_spmd

    x = np.asarray(x, dtype=np.float32)
    Wq1 = np.asarray(Wq1, dtype=np.float32)
    Wq2 = np.asarray(Wq2, dtype=np.float32)
    Wk1 = np.asarray(Wk1, dtype=np.float32)
    Wk2 = np.asarray(Wk2, dtype=np.float32)
    Wv1 = np.asarray(Wv1, dtype=np.float32)
    Wv2 = np.asarray(Wv2, dtype=np.float32)
    Wo = np.asarray(Wo, dtype=np.float32)

    if "nc" not in _CACHE:
        _CACHE["nc"] = _build_nc()
    nc = _CACHE["nc"]

    bf = ml_dtypes.bfloat16
    wq1_b = Wq1.astype(bf)
    wq2_b = (Wq2 / np.sqrt(DK)).astype(bf)
    wk1_b = Wk1.astype(bf)
    wk2_b = Wk2.astype(bf)
    wv1_b = Wv1.astype(bf)
    wv2_b = Wv2.astype(bf)
    wo_b = Wo.astype(bf)
    masks = {p: _make_mask(p).astype(bf) for p in range(2)}
    rows = {p: _rows_sched(p) for p in range(2)}
    ones_np = np.ones((P, P), bf)

    in_maps = []
    for core in range(8):
        batch, parity = core // 2, core % 2
        xb = x[batch]
        xbT = np.ascontiguousarray(xb.T)
        in_maps.append(
            {
                "xkv": np.ascontiguousarray(
                    xbT[:, parity * SH : (parity + 1) * SH]
                ).astype(bf),
                "xq": np.ascontiguousarray(xb[rows[parity]].T).astype(bf),
                "wq1": wq1_b,
                "wq2": wq2_b,
                "wk1": wk1_b,
                "wk2": wk2_b,
                "wv1": wv1_b,
                "wv2": wv2_b,
                "wo": wo_b,
                "maskin": masks[parity],
                "ones_in": ones_np,
            }
        )

    res = run_bass_kernel_spmd(nc, in_maps, core_ids=list(range(8)), trace=TRACE)
    LAST_RESULT = res

    out_full = np.empty((B, S, D), np.float32)
    for core in range(8):
        batch, parity = core // 2, core % 2
        out_full[batch][rows[parity]] = res.results[core]["out"]
    return out_full
